# revision 20
# baseline (speedup 1.0000x reference)
"""Trainium2 Bass kernel for a 2-layer GCN link-prediction model (DDI-style graph).

v2 — gpsimd-descriptor-floor architecture. The Q7 SWDGE descriptor loop
(~8ns/row) is the hard bottleneck, so everything is organized to (a) gather
the minimum number of rows and (b) keep the Pool engine generating
descriptors 100% of the time:

  - node table is split into two sub-tables A/B (by row range within each
    core's shard) so the inter-layer AllGather runs as two halves that
    overlap the conv tail;
  - per-(tile, subtable) edge sections are padded only to the cross-core
    max (SPMD needs a common instruction stream), not to 128-row chunks;
    chunk->tile mapping is resolved by two dstloc indicator variants;
  - gather streams interleave groups of 4 dst tiles (A-part then B-part) to
    bound live PSUM accumulators;
  - the first calls of every phase are PREPARE_ONLY descriptor preps fired
    by a trigger that waits for the table write/AllGather, so Q7 descriptor
    generation overlaps table production.
"""

import sys
import numpy as np
import ml_dtypes

sys.path.insert(0, "/opt/trn_rl_repo")

import concourse.bass as bass
import concourse.bacc as bacc
import concourse.mybir as mybir
import concourse.tile as tile
from concourse import bass_utils

BF16 = ml_dtypes.bfloat16

N_NODES = 50000
N_EDGES = 800000
N_QUERY = 200000
H = 128
NCLS = 86
P = 128
NCORES = 8
TPC = 49                  # dst tiles per core
NT = TPC * NCORES         # 392 global tiles (incl pad tile)
NPAD = NT * P             # 50176
SHARD = TPC * P           # 6272 rows per core
TA_T = 25                 # tiles per core in subtable A
TB_T = TPC - TA_T         # 24 in B
RA = TA_T * P             # 3200 rows/core in A
RB = TB_T * P             # 3072
NA = RA * NCORES          # 25600 rows in table A
NB = RB * NCORES          # 24576 rows in table B
G_T = 4                   # dst tiles per stream group
CALL = 6144               # gather rows per call (conv)
QCALL = 3072              # queries per decode gather call
QSL = 512                 # decode matmul slab

TRACE = False
RUN_KWARGS = {}
import os as _os
USE_PREP = _os.environ.get("USE_PREP", "0") == "1"
LAST_EXEC_NS = None
LAST_RESULTS = None


def _wrap_idx(idx_list):
    """dma_gather SBUF idx layout: element j at [j % 16, j // 16], replicated
    across the 8 Q7 core groups. [128, len/16] int16."""
    L = len(idx_list)
    assert L % 16 == 0
    base = np.asarray(idx_list, np.int16).reshape(L // 16, 16).T
    return np.tile(base, (8, 1))


def _ceil_div(a, b):
    return -(-a // b)


def _node_row(n):
    """Map node id -> (sub, row) in the A/B split tables."""
    c, r = n // SHARD, n % SHARD
    sub = (r >= RA).astype(np.int64)
    row = np.where(sub == 0, c * RA + r, c * RB + (r - RA))
    return sub, row


def _prep_conv(edge_index):
    """Build the conv gather schedule (shared by both layers).

    Stream layout per core: for each group of G_T dst tiles:
      [A-sections of the group's tiles | pad %128 | B-sections | pad %128]
    Section length per (tile, sub) = max over cores (common SPMD schedule).
    Returns (sched, per_core idx/dstloc arrays, deg).
    """
    src = np.asarray(edge_index[0], np.int64)
    dst = np.asarray(edge_index[1], np.int64)
    self_ids = np.arange(N_NODES, dtype=np.int64)
    src = np.concatenate([src, self_ids])
    dst = np.concatenate([dst, self_ids])

    deg = np.bincount(dst, minlength=NPAD).astype(np.float32)
    deg[N_NODES:] = 1.0

    ssub, srow = _node_row(src)
    t = dst // P
    co = t // TPC
    lt = t % TPC
    dloc = dst % P

    # bucket edges by (core, lt, sub); sort each bucket by src row
    ord_ = np.lexsort((srow, ssub, lt, co))
    co_s, lt_s, sub_s, row_s, dl_s = co[ord_], lt[ord_], ssub[ord_], srow[ord_], dloc[ord_]
    key = ((co_s * TPC + lt_s) * 2 + sub_s)
    bnd = np.searchsorted(key, np.arange(NCORES * TPC * 2 + 1))
    cnt = (bnd[1:] - bnd[:-1]).reshape(NCORES, TPC, 2)
    L = cnt.max(axis=0)  # [TPC, 2] cross-core max section lengths

    # stream layout (common across cores): [all A-sections | all B-sections]
    sections = []   # (lt, sub, off, len)
    calls = []      # dicts: sub, row0, nrows, chunk0, nch
    off = 0
    for sub in (0, 1):
        p0 = off
        for lt_i in range(TPC):
            sections.append((lt_i, sub, off, int(L[lt_i, sub])))
            off += int(L[lt_i, sub])
        off = _ceil_div(off, P) * P  # pad part to chunk boundary
        r = p0
        while r < off:
            n = min(CALL, off - r)
            calls.append({"sub": sub, "row0": r, "nrows": n,
                          "chunk0": r // P, "nch": n // P})
            r += n
    TOT = off
    NCH = TOT // P

    # per-chunk variants: list of (v, lt, start, stop) ; dstloc variant arrays
    chunk_var = [[] for _ in range(NCH)]
    for (lt_i, sub, soff, slen) in sections:
        if slen == 0:
            continue
        c0, c1 = soff // P, (soff + slen - 1) // P
        for ch in range(c0, c1 + 1):
            v = len(chunk_var[ch])
            assert v < 2, "more than 2 tile-sections in one chunk"
            chunk_var[ch].append([v, lt_i, False, False])
    # start/stop per (tile, pass): chunk_var entries don't know sub, so
    # recover it from the chunk offset (A chunks come first)
    nchA = next(cl["chunk0"] + cl["nch"] for cl in reversed(calls) if cl["sub"] == 0)
    tile_pieces = {}
    for ch in range(NCH):
        for ent in chunk_var[ch]:
            sub = 0 if ch < nchA else 1
            tile_pieces.setdefault((ent[1], sub), []).append((ch, ent))
    for ps in tile_pieces.values():
        ps[0][1][2] = True
        ps[-1][1][3] = True

    # per-call matmul schedule
    for cl in calls:
        sched_mm = []
        for chl in range(cl["nch"]):
            ch = cl["chunk0"] + chl
            for (v, lt_i, st, sp) in chunk_var[ch]:
                sched_mm.append((chl, v, lt_i, st, sp, cl["sub"]))
        cl["mm"] = sched_mm
        cl["has_v1"] = any(v == 1 for (_, v, _, _, _, _) in cl["mm"])

    sched = {"TOT": TOT, "NCH": NCH, "calls": calls}

    # per-core data
    per_core = []
    for c in range(NCORES):
        idx = np.zeros(TOT, np.int16)
        dl0 = np.full((P, NCH), 255.0, BF16)
        dl1 = np.full((P, NCH), 255.0, BF16)
        for (lt_i, sub, soff, slen) in sections:
            b = bnd[((c * TPC + lt_i) * 2 + sub)]
            e = bnd[((c * TPC + lt_i) * 2 + sub) + 1]
            n = e - b
            assert n <= slen
            idx[soff : soff + n] = row_s[b:e].astype(np.int16)
            dls = np.full(slen, 255.0, BF16)
            dls[:n] = dl_s[b:e].astype(BF16)
            # scatter into dl0/dl1 by variant
            c0 = soff // P
            for ch in range(c0, (soff + slen - 1) // P + 1):
                v = next(vv for (vv, l2, _, _) in chunk_var[ch] if l2 == lt_i)
                a0 = max(soff, ch * P)
                a1 = min(soff + slen, (ch + 1) * P)
                tgt = dl0 if v == 0 else dl1
                tgt[a0 - ch * P : a1 - ch * P, ch] = dls[a0 - soff : a1 - soff]
        per_core.append({"cidx": _wrap_idx(idx), "dl0": dl0, "dl1": dl1})

    return sched, per_core, deg


def _prep_decode(edge_label_index):
    """Shard queries across cores; sort into 4 groups by (sub_a, sub_b);
    pad groups to cross-core max (%128). Calls of <= QCALL queries."""
    a = np.asarray(edge_label_index[0], np.int64)
    b = np.asarray(edge_label_index[1], np.int64)
    qpc = N_QUERY // NCORES
    asub, arow = _node_row(a)
    bsub, brow = _node_row(b)
    core_groups = []
    for c in range(NCORES):
        sl = slice(c * qpc, (c + 1) * qpc)
        key = asub[sl] * 2 + bsub[sl]
        gidx = [np.nonzero(key == k)[0] for k in range(4)]
        core_groups.append(gidx)
    G = [_ceil_div(max(len(core_groups[c][k]) for c in range(NCORES)), P) * P
         for k in range(4)]
    QPAD = sum(G)

    # calls (common layout): within each group, pieces of <= QCALL
    calls = []
    off = 0
    for k in range(4):
        r = 0
        while r < G[k]:
            n = min(QCALL, G[k] - r)
            calls.append({"suba": k // 2, "subb": k % 2, "q0": off + r, "nq": n})
            r += n
        off += G[k]

    per_core = []
    perms = []
    for c in range(NCORES):
        sl = slice(c * qpc, (c + 1) * qpc)
        ar, br = arow[sl], brow[sl]
        qa = np.zeros(QPAD, np.int16)
        qb = np.zeros(QPAD, np.int16)
        perm = np.full(QPAD, -1, np.int64)
        off = 0
        for k in range(4):
            ids = core_groups[c][k]
            qa[off : off + len(ids)] = ar[ids].astype(np.int16)
            qb[off : off + len(ids)] = br[ids].astype(np.int16)
            perm[off : off + len(ids)] = c * qpc + ids
            off += G[k]
        per_core.append({"qa": _wrap_idx(qa), "qb": _wrap_idx(qb)})
        perms.append(perm)
    dec = {"G": G, "QPAD": QPAD, "calls": calls}
    return dec, per_core, perms


def _build(sched, dec):
    nc = bacc.Bacc("TRN2", target_bir_lowering=False, debug=False,
                   num_devices=NCORES,
                   dynamic_dma_scratch_size=int(_os.environ.get("DDSS", "16384")))
    f32, bf16, i16 = mybir.dt.float32, mybir.dt.bfloat16, mybir.dt.int16
    AF = mybir.ActivationFunctionType
    ALU = mybir.AluOpType

    TOT, NCH, calls = sched["TOT"], sched["NCH"], sched["calls"]
    QPAD, qcalls = dec["QPAD"], dec["calls"]

    # ---- I/O ----
    embs_in = nc.dram_tensor("emb_shard", [SHARD, H], f32, kind="ExternalInput").ap()
    degs_in = nc.dram_tensor("deg_s", [P, TPC], f32, kind="ExternalInput").ap()
    w1_in = nc.dram_tensor("w1", [H, H], f32, kind="ExternalInput").ap()
    w2_in = nc.dram_tensor("w2", [H, H], f32, kind="ExternalInput").ap()
    b1_in = nc.dram_tensor("b1", [1, H], f32, kind="ExternalInput").ap()
    b2_in = nc.dram_tensor("b2", [1, H], f32, kind="ExternalInput").ap()
    dw1t_in = nc.dram_tensor("dw1t", [H, H], f32, kind="ExternalInput").ap()
    dw1b_in = nc.dram_tensor("dw1b", [H, H], f32, kind="ExternalInput").ap()
    db1_in = nc.dram_tensor("db1", [H, 1], f32, kind="ExternalInput").ap()
    dw2_in = nc.dram_tensor("dw2", [H, NCLS], f32, kind="ExternalInput").ap()
    db2_in = nc.dram_tensor("db2", [NCLS, 1], f32, kind="ExternalInput").ap()
    cidx_in = nc.dram_tensor("cidx", [P, TOT // 16], i16, kind="ExternalInput").ap()
    dl0_in = nc.dram_tensor("dl0", [P, NCH], bf16, kind="ExternalInput").ap()
    dl1_in = nc.dram_tensor("dl1", [P, NCH], bf16, kind="ExternalInput").ap()
    qa_in = nc.dram_tensor("qa", [P, QPAD // 16], i16, kind="ExternalInput").ap()
    qb_in = nc.dram_tensor("qb", [P, QPAD // 16], i16, kind="ExternalInput").ap()
    logits_out = nc.dram_tensor("logitsT", [NCLS, QPAD], f32, kind="ExternalOutput").ap()

    # ---- internal DRAM ----
    hA_sh = nc.dram_tensor("hA_sh", [RA, H], bf16).ap()
    hB_sh = nc.dram_tensor("hB_sh", [RB, H], bf16).ap()
    hA = nc.dram_tensor("hA", [NA, H], bf16, addr_space="Shared").ap()
    hB = nc.dram_tensor("hB", [NB, H], bf16, addr_space="Shared").ap()
    gA_sh = nc.dram_tensor("gA_sh", [RA, H], bf16).ap()
    gB_sh = nc.dram_tensor("gB_sh", [RB, H], bf16).ap()
    gA = nc.dram_tensor("gA", [NA, H], bf16, addr_space="Shared").ap()
    gB = nc.dram_tensor("gB", [NB, H], bf16, addr_space="Shared").ap()
    zA_sh = nc.dram_tensor("zA_sh", [RA, H], bf16).ap()
    zB_sh = nc.dram_tensor("zB_sh", [RB, H], bf16).ap()
    zA = nc.dram_tensor("zA", [NA, H], bf16, addr_space="Shared").ap()
    zB = nc.dram_tensor("zB", [NB, H], bf16, addr_space="Shared").ap()

    ident_np = np.eye(P, dtype=BF16)
    iota_np = np.tile(np.arange(P, dtype=BF16)[None, :], (P, 1))
    ones_np = np.ones((1, P), dtype=BF16)
    ident_c = nc.inline_tensor(ident_np, "ident_c").ap()
    iota_c = nc.inline_tensor(iota_np, "iota_c").ap()
    ones_c = nc.inline_tensor(ones_np, "ones_c").ap()

    rg = [list(range(NCORES))]
    swsems = []

    def next_sem():
        if len(swsems) < 8:
            swsems.append(nc.alloc_semaphore(f"swg{len(swsems)}"))
            return swsems[-1]
        return swsems[next_sem.i % 8]


    next_sem.i = 0

    def next_sem_rr():
        sem = swsems[next_sem.i % 8] if len(swsems) >= 8 else None
        if sem is None:
            sem = nc.alloc_semaphore(f"swg{len(swsems)}")
            swsems.append(sem)
        next_sem.i += 1
        return sem

    with tile.TileContext(nc, trace_sim=False) as tc:
        import contextlib
        ctx = contextlib.ExitStack()
        with ctx:
            cpool = ctx.enter_context(tc.tile_pool(name="consts", bufs=1))
            epool = ctx.enter_context(tc.tile_pool(name="embbuild", bufs=3))
            gpool = ctx.enter_context(tc.tile_pool(name="gather", bufs=2))
            ipool = ctx.enter_context(tc.tile_pool(name="indic", bufs=2))
            i1pool = ctx.enter_context(tc.tile_pool(name="indic1", bufs=2))
            spool = ctx.enter_context(tc.tile_pool(name="small", bufs=3))
            qpool = ctx.enter_context(tc.tile_pool(name="dec", bufs=3))


            ident = cpool.tile([P, P], bf16, tag="ident")
            nc.sync.dma_start(ident[:], ident_c[:])
            iota = cpool.tile([P, P], bf16, tag="iota")
            nc.sync.dma_start(iota[:], iota_c[:])
            ones1 = cpool.tile([1, P], bf16, tag="ones1")
            nc.sync.dma_start(ones1[:], ones_c[:])

            def load_bf(ap_in, shape, tag):
                tf = cpool.tile(shape, f32, tag=tag + "_f")
                nc.sync.dma_start(tf[:], ap_in[:])
                tb = cpool.tile(shape, bf16, tag=tag)
                nc.vector.tensor_copy(tb[:], tf[:])
                return tb

            w1 = load_bf(w1_in, [H, H], "w1")
            w2 = load_bf(w2_in, [H, H], "w2")
            b1r = load_bf(b1_in, [1, H], "b1r")
            b2r = load_bf(b2_in, [1, H], "b2r")
            dw1t = load_bf(dw1t_in, [H, H], "dw1t")
            dw1b = load_bf(dw1b_in, [H, H], "dw1b")
            dw2 = load_bf(dw2_in, [H, NCLS], "dw2")
            db1 = cpool.tile([H, 1], f32, tag="db1")
            nc.sync.dma_start(db1[:], db1_in[:])
            db2 = cpool.tile([NCLS, 1], f32, tag="db2")
            nc.sync.dma_start(db2[:], db2_in[:])

            degs = cpool.tile([P, TPC], f32, tag="degs")
            nc.sync.dma_start(degs[:], degs_in[:])
            recs = cpool.tile([P, TPC], f32, tag="recs")
            nc.vector.reciprocal(recs[:], degs[:])
            dis_sh = cpool.tile([P, TPC], f32, tag="dis_sh")
            nc.scalar.sqrt(dis_sh[:], recs[:])

            cidx = cpool.tile([P, TOT // 16], i16, tag="cidx")
            nc.sync.dma_start(cidx[:], cidx_in[:])
            dl0 = cpool.tile([P, NCH], bf16, tag="dl0")
            nc.sync.dma_start(dl0[:], dl0_in[:])
            dl1 = cpool.tile([P, NCH], bf16, tag="dl1")
            nc.sync.dma_start(dl1[:], dl1_in[:])
            qa_sb = cpool.tile([P, QPAD // 16], i16, tag="qa")
            nc.sync.dma_start(qa_sb[:], qa_in[:])
            qb_sb = cpool.tile([P, QPAD // 16], i16, tag="qb")
            nc.sync.dma_start(qb_sb[:], qb_in[:])

            # ---------- phase 1: h' = dis * emb, sharded; AG halves ----------
            BT = 5
            for t0 in range(0, TPC, BT):
                tn = min(BT, TPC - t0)
                et = epool.tile([P, BT * H], f32, tag="et")
                nc.sync.dma_start(
                    et[:, : tn * H].rearrange("p (c e) -> p c e", e=H),
                    embs_in[t0 * P : (t0 + tn) * P, :].rearrange("(c p) e -> p c e", p=P),
                )
                ep = epool.tile([P, BT * H], bf16, tag="ep")
                dis_b = dis_sh[:, t0 : t0 + tn].unsqueeze(2).to_broadcast([P, tn, H])
                nc.vector.tensor_tensor(
                    ep[:, : tn * H].rearrange("p (c e) -> p c e", e=H),
                    et[:, : tn * H].rearrange("p (c e) -> p c e", e=H),
                    dis_b,
                    op=ALU.mult,
                )
                # rows [t0*P, (t0+tn)*P) split across hA_sh / hB_sh
                r0, r1 = t0 * P, (t0 + tn) * P
                if r0 < RA:
                    n = min(r1, RA) - r0
                    nc.sync.dma_start(
                        hA_sh[r0 : r0 + n, :].rearrange("(c p) e -> p c e", p=P),
                        ep[:, : n // P * H].rearrange("p (c e) -> p c e", e=H),
                    )
                if r1 > RA:
                    s0 = max(r0, RA)
                    k0 = (s0 - r0) // P
                    n = r1 - s0
                    nc.sync.dma_start(
                        hB_sh[s0 - RA : s0 - RA + n, :].rearrange("(c p) e -> p c e", p=P),
                        ep[:, k0 * H : (k0 + n // P) * H].rearrange("p (c e) -> p c e", e=H),
                    )
            nc.gpsimd.collective_compute(
                "AllGather", ALU.bypass, ins=[hA_sh[:]], outs=[hA[:]], replica_groups=rg)
            nc.gpsimd.collective_compute(
                "AllGather", ALU.bypass, ins=[hB_sh[:]], outs=[hB[:]], replica_groups=rg)

            # ---------- conv layer ----------
            UA = cpool.tile([P, TPC * H], f32, tag="UA")

            def conv_layer(tabA, tabB, outA_sh, outB_sh, w, brow, is_first, lname,
                           pp_u, pp_e):
                u_ps = {}
                nc.vector.memset(UA[:], 0.0)
                for ci, cl in enumerate(calls):
                    tab = tabA if cl["sub"] == 0 else tabB
                    gb = gpool.tile([P, CALL // P * H], bf16, tag="gb")
                    kw = {}
                    if USE_PREP and ci < 2:
                        kw = dict(prepare_only=True, sem=next_sem_rr())
                    nc.gpsimd.dma_gather(
                        out_ap=gb[:, : cl["nch"] * H].rearrange("p (c e) -> p c e", e=H),
                        in_ap=tab,
                        idxs_ap=cidx[:, cl["row0"] // 16 : (cl["row0"] + cl["nrows"]) // 16],
                        num_idxs=cl["nrows"],
                        num_idxs_reg=cl["nrows"],
                        elem_size=H,
                        single_packet=False,
                        **kw,
                    )
                    if USE_PREP and ci == 1:
                        nc.gpsimd.trigger_dma(count=None)
                    nch = cl["nch"]
                    ch0 = cl["chunk0"]
                    ind0 = ipool.tile([P, CALL // P * H], bf16, tag="ind0")
                    nc.vector.tensor_tensor(
                        ind0[:, : nch * H].rearrange("p (c e) -> p c e", e=H),
                        iota[:].unsqueeze(1).to_broadcast([P, nch, H]),
                        dl0[:, ch0 : ch0 + nch].unsqueeze(2).to_broadcast([P, nch, H]),
                        op=ALU.is_equal,
                    )
                    if cl["has_v1"]:
                        ind1 = i1pool.tile([P, CALL // P * H], bf16, tag="ind1")
                        nc.vector.tensor_tensor(
                            ind1[:, : nch * H].rearrange("p (c e) -> p c e", e=H),
                            iota[:].unsqueeze(1).to_broadcast([P, nch, H]),
                            dl1[:, ch0 : ch0 + nch].unsqueeze(2).to_broadcast([P, nch, H]),
                            op=ALU.is_equal,
                        )
                    for (chl, v, lt_i, st, sp, sub) in cl["mm"]:
                        ind = ind0 if v == 0 else ind1
                        if st:
                            u_acc = pp_u.tile([P, H], f32, tag="u")
                            u_ps[lt_i] = u_acc
                        nc.tensor.matmul(
                            out=u_ps[lt_i][:],
                            lhsT=ind[:, chl * H : (chl + 1) * H],
                            rhs=gb[:, chl * H : (chl + 1) * H],
                            start=st,
                            stop=sp,
                        )
                        if sp and sub == 0:
                            # stash A-pass partial sum in SBUF f32
                            ups = u_ps.pop(lt_i)
                            nc.scalar.copy(UA[:, lt_i * H : (lt_i + 1) * H], ups[:])
                        elif sp:
                            # epilogue: U = UA + UB; VT = (dis*U)^T; z = V@W + b
                            ups = u_ps.pop(lt_i)
                            u_bf = spool.tile([P, H], bf16, tag="u_bf")
                            nc.vector.tensor_tensor(
                                u_bf[:], UA[:, lt_i * H : (lt_i + 1) * H], ups[:],
                                op=ALU.add)
                            diag = spool.tile([P, H], bf16, tag="diag")
                            nc.scalar.mul(diag[:], ident[:], mul=dis_sh[:, lt_i : lt_i + 1])
                            vt_ps = pp_e.tile([P, H], f32, tag="e")
                            nc.tensor.matmul(out=vt_ps[:], lhsT=u_bf[:], rhs=diag[:],
                                             start=True, stop=True)
                            vt_bf = spool.tile([P, H], bf16, tag="vt_bf")
                            nc.scalar.copy(vt_bf[:], vt_ps[:])
                            z_ps = pp_e.tile([P, H], f32, tag="e")
                            nc.tensor.matmul(out=z_ps[:], lhsT=ones1[:], rhs=brow[:],
                                             start=True, stop=False)
                            nc.tensor.matmul(out=z_ps[:], lhsT=vt_bf[:], rhs=w[:],
                                             start=False, stop=True)
                            o_bf = spool.tile([P, H], bf16, tag="o_bf")
                            if is_first:
                                nc.scalar.activation(o_bf[:], z_ps[:], AF.Relu,
                                                     scale=dis_sh[:, lt_i : lt_i + 1])
                            else:
                                nc.scalar.copy(o_bf[:], z_ps[:])
                            if lt_i < TA_T:
                                nc.sync.dma_start(
                                    outA_sh[lt_i * P : (lt_i + 1) * P, :], o_bf[:])
                            else:
                                nc.sync.dma_start(
                                    outB_sh[(lt_i - TA_T) * P : (lt_i - TA_T + 1) * P, :],
                                    o_bf[:])
                assert not u_ps

            with tc.tile_pool(name="ps_u", bufs=3, space="PSUM") as pp_u, \
                 tc.tile_pool(name="ps_e", bufs=1, space="PSUM") as pp_e:
                conv_layer(hA, hB, gA_sh, gB_sh, w1, b1r, True, "L1", pp_u, pp_e)
                nc.gpsimd.collective_compute(
                    "AllGather", ALU.bypass, ins=[gA_sh[:]], outs=[gA[:]], replica_groups=rg)
                nc.gpsimd.collective_compute(
                    "AllGather", ALU.bypass, ins=[gB_sh[:]], outs=[gB[:]], replica_groups=rg)
                conv_layer(gA, gB, zA_sh, zB_sh, w2, b2r, False, "L2", pp_u, pp_e)
                nc.gpsimd.collective_compute(
                    "AllGather", ALU.bypass, ins=[zA_sh[:]], outs=[zA[:]], replica_groups=rg)
                nc.gpsimd.collective_compute(
                    "AllGather", ALU.bypass, ins=[zB_sh[:]], outs=[zB[:]], replica_groups=rg)

            # ---------- decode ----------
            pp_d = ctx.enter_context(tc.tile_pool(name="ps_d", bufs=2, space="PSUM"))
            for qi, ql in enumerate(qcalls):
                a_tab = zA if ql["suba"] == 0 else zB
                b_tab = zA if ql["subb"] == 0 else zB
                nq, q0 = ql["nq"], ql["q0"]
                zaT = qpool.tile([P, QCALL], bf16, tag="zaT")
                kwa = {}
                kwb = {}
                if USE_PREP and qi == 0:
                    kwa = dict(prepare_only=True, sem=next_sem_rr())
                    kwb = dict(prepare_only=True, sem=next_sem_rr())
                nc.gpsimd.dma_gather(
                    out_ap=zaT[:, :nq].rearrange("p (c q) -> p c q", c=1),
                    in_ap=a_tab,
                    idxs_ap=qa_sb[:, q0 // 16 : (q0 + nq) // 16],
                    num_idxs=nq, num_idxs_reg=nq, elem_size=H,
                    transpose=True, single_packet=False, **kwa,
                )
                zbT = qpool.tile([P, QCALL], bf16, tag="zbT")
                nc.gpsimd.dma_gather(
                    out_ap=zbT[:, :nq].rearrange("p (c q) -> p c q", c=1),
                    in_ap=b_tab,
                    idxs_ap=qb_sb[:, q0 // 16 : (q0 + nq) // 16],
                    num_idxs=nq, num_idxs_reg=nq, elem_size=H,
                    transpose=True, single_packet=False, **kwb,
                )
                if USE_PREP and qi == 0:
                    nc.gpsimd.trigger_dma(count=None)
                for s0 in range(0, nq, QSL):
                    sn = min(QSL, nq - s0)
                    h_ps = pp_d.tile([P, QSL], f32, tag="h")
                    nc.tensor.matmul(out=h_ps[:, :sn], lhsT=dw1t[:],
                                     rhs=zaT[:, s0 : s0 + sn], start=True, stop=False)
                    nc.tensor.matmul(out=h_ps[:, :sn], lhsT=dw1b[:],
                                     rhs=zbT[:, s0 : s0 + sn], start=False, stop=True)
                    hT = qpool.tile([P, QSL], bf16, tag="hT")
                    nc.scalar.activation(hT[:, :sn], h_ps[:, :sn], AF.Relu, bias=db1[:])
                    l_ps = pp_d.tile([NCLS, QSL], f32, tag="l")
                    nc.tensor.matmul(out=l_ps[:, :sn], lhsT=dw2[:], rhs=hT[:, :sn],
                                     start=True, stop=True)
                    lf = qpool.tile([NCLS, QSL], f32, tag="lf")
                    nc.scalar.activation(lf[:, :sn], l_ps[:, :sn], AF.Identity, bias=db2[:])
                    nc.sync.dma_start(logits_out[:, q0 + s0 : q0 + s0 + sn], lf[:, :sn])

    nc.compile()
    return nc


def kernel(**inputs):
    emb = np.asarray(inputs["emb"], np.float32)
    x = np.asarray(inputs["x"], np.int64)
    if not np.array_equal(x, np.arange(N_NODES)):
        emb = emb[x]
    embp = np.zeros((NPAD, H), np.float32)
    embp[:N_NODES] = emb

    sched, conv_pc, deg = _prep_conv(np.asarray(inputs["edge_index"], np.int64))
    dec, dec_pc, perms = _prep_decode(np.asarray(inputs["edge_label_index"], np.int64))

    nc = _build(sched, dec)

    dW1 = np.asarray(inputs["dW1"], np.float32)
    deg_rt = deg.reshape(NT, P).T  # [P, NT]
    in_maps = []
    for c in range(NCORES):
        t0 = c * TPC
        in_maps.append({
            "emb_shard": np.ascontiguousarray(embp[c * SHARD : (c + 1) * SHARD]),
            "deg_s": np.ascontiguousarray(deg_rt[:, t0 : t0 + TPC]),
            "w1": np.asarray(inputs["W1"], np.float32),
            "w2": np.asarray(inputs["W2"], np.float32),
            "b1": np.asarray(inputs["b1"], np.float32).reshape(1, H),
            "b2": np.asarray(inputs["b2"], np.float32).reshape(1, H),
            "dw1t": np.ascontiguousarray(dW1[:H]),
            "dw1b": np.ascontiguousarray(dW1[H:]),
            "db1": np.asarray(inputs["db1"], np.float32).reshape(H, 1),
            "dw2": np.asarray(inputs["dW2"], np.float32),
            "db2": np.asarray(inputs["db2"], np.float32).reshape(NCLS, 1),
            "cidx": conv_pc[c]["cidx"],
            "dl0": conv_pc[c]["dl0"],
            "dl1": conv_pc[c]["dl1"],
            "qa": dec_pc[c]["qa"],
            "qb": dec_pc[c]["qb"],
        })

    res = bass_utils.run_bass_kernel_spmd(
        nc, in_maps, core_ids=list(range(NCORES)), trace=TRACE, **RUN_KWARGS
    )
    globals()["LAST_EXEC_NS"] = res.exec_time_ns
    globals()["LAST_RESULTS"] = res

    out = np.zeros((N_QUERY, NCLS), np.float32)
    for c in range(NCORES):
        lt = np.asarray(res.results[c]["logitsT"], np.float32).T  # [QPAD, NCLS]
        perm = perms[c]
        m = perm >= 0
        out[perm[m]] = lt[m]
    return out


def _sim_conv_check(edge_index):
    """Host-side schedule validator: simulate the gather+indicator+matmul
    pipeline in numpy against a direct segment-sum."""
    sched, per_core, deg = _prep_conv(edge_index)
    rngv = np.random.default_rng(1)
    tabA = rngv.standard_normal((NA, H)).astype(np.float32)
    tabB = rngv.standard_normal((NB, H)).astype(np.float32)
    src = np.asarray(edge_index[0], np.int64)
    dst = np.asarray(edge_index[1], np.int64)
    self_ids = np.arange(N_NODES, dtype=np.int64)
    src = np.concatenate([src, self_ids])
    dst = np.concatenate([dst, self_ids])
    ssub, srow = _node_row(src)
    rows = np.where(ssub == 0, tabA[srow % NA].T, tabB[srow % NB].T).T  # careful
    # direct U
    U_ref = np.zeros((NPAD, H), np.float32)
    np.add.at(U_ref, dst, rows)

    ok = True
    for c in range(NCORES):
        pc = per_core[c]
        idx = np.asarray([pc["cidx"][j % 16, j // 16] for j in range(sched["TOT"])],
                         np.int64)
        U = np.zeros((TPC, P, H), np.float32)
        for cl in sched["calls"]:
            tab = tabA if cl["sub"] == 0 else tabB
            gb = tab[idx[cl["row0"] : cl["row0"] + cl["nrows"]]]  # [n, H]
            gb = gb.reshape(cl["nch"], P, H)
            for (chl, v, lt_i, st, sp, sub) in cl["mm"]:
                dlv = pc["dl0"] if v == 0 else pc["dl1"]
                dl_col = np.asarray(dlv[:, cl["chunk0"] + chl], np.float32)
                ind = (dl_col[:, None] == np.arange(P)[None, :]).astype(np.float32)
                U[lt_i] += ind.T @ gb[chl]
        exp = U_ref[c * SHARD : (c + 1) * SHARD].reshape(TPC, P, H)
        e = np.abs(U - exp).max()
        if e > 1e-3:
            print(f"core {c}: conv sim mismatch {e}")
            ok = False
    print("conv schedule sim:", "OK" if ok else "FAIL")
    return ok


if __name__ == "__main__":
    rng = np.random.default_rng(0)
    ei = rng.integers(0, N_NODES, (2, N_EDGES))
    _sim_conv_check(ei)
